# revision 1
# baseline (speedup 1.0000x reference)
"""Trainium2 Bass kernel for nn_ClassAtt (dense MLP + 3-way class attention).

Model (per row of tube [B, 1536]):
  x1,x2,x3 = tube split into 3x512
  P_i   = relu(x_i @ w_i.T + b_i)            [B, 1024]
  last  = relu(concat(P1,P2,P3) @ wh.T + bh) [B, 1024]
  a_i   = rowwise_dot(last, P_i); w = softmax(a)  [B, 3]
  ctx   = sum_i w_i * P_i                    [B, 1024]
  out   = relu(concat(ctx, last) @ wd1.T + bd1) @ wd2.T + bd2  [B, 1000]

Strategy: pure data parallel over 8 NeuronCores (2048 rows each).  All
activations live in transposed [feature, row] layout so the contraction dim
is always on SBUF partitions and biases are per-partition scalars.  Weights
are host-transposed to [K, F].  Matmuls run as float32r (full PE rate for
free dim >= 256, ~1e-4 relative rounding) with fp32 PSUM accumulation.
Phases (DRAM spills between them):
  P1: L1 (3x 512->1024) -> PT spill   [whT chunk-loads interleaved into P1]
  P2: L2 (3072->1024) + attention -> dec spill (= [ctx; last])
  F0/F1: decode split by contraction half: out_h = relu(dec @ wd1_h.T
        + bd1_h) @ wd2_h.T; host sums the two halves (+bd2 in half 1).
DMA ordering matters: weight tensors are loaded with per-chunk DMAs
interleaved after the activation loads they must not delay (HWDGE is FIFO
per issuing engine).  The attention elementwise products run on GpSimd to
keep VectorE off the critical path.
"""

import numpy as np

import concourse.bass as bass
import concourse.mybir as mybir
import concourse.tile as tile
from concourse import bacc
from concourse.bass_utils import run_bass_kernel_spmd

F32 = mybir.dt.float32
F32R = mybir.dt.float32r

N_CORES = 8
B = 16384
ROWS = B // N_CORES  # rows per core
M = 1024             # hidden width
DEC_H = 2048
OUT = 1000

AluOp = mybir.AluOpType
Act = mybir.ActivationFunctionType


def build_nc(mm_dtype=F32R):
    nc = bacc.Bacc(None, target_bir_lowering=False)

    # ---- DRAM I/O (per-core shapes) ----
    xT = nc.dram_tensor("xT", [12, 128, ROWS], mm_dtype, kind="ExternalInput")
    wT = [
        nc.dram_tensor(f"w{i + 1}T", [4, 128, M], mm_dtype, kind="ExternalInput")
        for i in range(3)
    ]
    whT = nc.dram_tensor("whT", [24, 128, M], mm_dtype, kind="ExternalInput")
    wd1T = nc.dram_tensor("wd1T", [16, 128, DEC_H], mm_dtype, kind="ExternalInput")
    wd2T = nc.dram_tensor("wd2T", [16, 128, OUT], mm_dtype, kind="ExternalInput")
    bv = [
        nc.dram_tensor(f"b{i + 1}", [128, 8], F32, kind="ExternalInput")
        for i in range(3)
    ]
    bh = nc.dram_tensor("bh", [128, 8], F32, kind="ExternalInput")
    bd1 = nc.dram_tensor("bd1", [128, 16], F32, kind="ExternalInput")
    bd2 = nc.dram_tensor("bd2", [128, 8], F32, kind="ExternalInput")
    outH = [
        nc.dram_tensor(f"out{h}", [OUT, ROWS], F32, kind="ExternalOutput")
        for h in range(2)
    ]

    with tile.TileContext(nc) as tc:
        with tc.tile_pool(name="dram", bufs=1, space="DRAM") as dram:
            PT = dram.tile([3, 8, 128, ROWS], mm_dtype)  # P_i transposed
            dec = dram.tile([8, 128, ROWS], mm_dtype)    # last, transposed
            WS = dram.tile([128, 3, ROWS], mm_dtype)     # softmax weights

            # p2w outlives phase 1 so whT streams in during P1's compute.
            with (
                tc.tile_pool(name="p2w", bufs=1) as p2w,
                tc.tile_pool(name="psA", bufs=3, space="PSUM") as psA,
            ):
                wh_sb = p2w.tile([128, 24, M], mm_dtype)
                bh_sb = p2w.tile([128, 8], F32, tag="bh")
                ones_f32 = p2w.tile([128, 128], F32, tag="ones_f32")
                ones_sb = p2w.tile([128, 128], mm_dtype, tag="ones")

                # ------------ Phase 1: P_i = relu(x_i @ w_i.T + b_i) --------
                R1 = 256
                NT1 = ROWS // R1
                with (
                    tc.tile_pool(name="p1w", bufs=1) as p1w,
                    tc.tile_pool(name="p1x", bufs=2) as p1x,
                    tc.tile_pool(name="p1e", bufs=3) as p1e,
                ):
                    # xt tiles created lazily, loads interleaved with weights
                    xts = {}

                    def load_xt(rt):
                        rs = slice(rt * R1, (rt + 1) * R1)
                        t = p1x.tile([128, 12, R1], mm_dtype, tag="xt",
                                     name="xt")
                        for i in range(3):
                            nc.sync.dma_start(
                                t[:, 4 * i:4 * i + 4, :],
                                xT.ap()[4 * i:4 * i + 4, :, rs]
                                .rearrange("c p r -> p c r"),
                            )
                        xts[rt] = t

                    w_sb = []
                    b_sb = []
                    for i in range(3):
                        w = p1w.tile([128, 4, M], mm_dtype, tag=f"w{i}",
                                     name=f"w{i}")
                        nc.scalar.dma_start(w, wT[i].ap().rearrange("c p f -> p c f"))
                        b = p1w.tile([128, 8], F32, tag=f"b{i}", name=f"b{i}")
                        nc.scalar.dma_start(b, bv[i].ap())
                        w_sb.append(w)
                        b_sb.append(b)
                        if i < 2:
                            load_xt(i)  # first row-tiles right behind w1
                    nc.scalar.dma_start(bh_sb, bh.ap())
                    nc.any.memset(ones_f32, 1.0)
                    nc.vector.tensor_copy(ones_sb, ones_f32)

                    for rt in range(NT1):
                        # stream 3 whT chunks per row-tile behind xt prefetch
                        if rt + 2 < NT1:
                            load_xt(rt + 2)
                        for c in range(3 * rt, 3 * rt + 3):
                            nc.scalar.dma_start(wh_sb[:, c, :], whT.ap()[c])
                        rs = slice(rt * R1, (rt + 1) * R1)
                        xt = xts.pop(rt)
                        for i in range(3):
                            ev = p1e.tile([128, 8, R1], mm_dtype)
                            for fc in range(8):
                                ps = psA.tile([128, R1], F32, tag="mm",
                                              name="ps1")
                                for kc in range(4):
                                    nc.tensor.matmul(
                                        ps,
                                        w_sb[i][:, kc, fc * 128:(fc + 1) * 128],
                                        xt[:, i * 4 + kc, :],
                                        start=(kc == 0),
                                        stop=(kc == 3),
                                    )
                                nc.vector.tensor_scalar(
                                    ev[:, fc, :], ps, b_sb[i][:, fc:fc + 1],
                                    0.0, AluOp.add, AluOp.max,
                                )
                            nc.sync.dma_start(
                                PT[i, :, :, rs].rearrange("c p r -> p c r"), ev
                            )

                # ------ Phase 2: last = relu(hid1 @ wh.T + bh); attention ---
                R2 = 256
                with (
                    tc.tile_pool(name="p2pt", bufs=2) as p2pt,
                    tc.tile_pool(name="p2last", bufs=2) as p2last,
                    tc.tile_pool(name="p2big", bufs=1) as p2big,
                    tc.tile_pool(name="p2sm", bufs=1) as p2sm,
                    tc.tile_pool(name="psC", bufs=5, space="PSUM") as psC,
                ):
                    for rt in range(ROWS // R2):
                        rs = slice(rt * R2, (rt + 1) * R2)
                        pt = []
                        for i in range(3):
                            pt_i = p2pt.tile([128, 8, R2], mm_dtype,
                                             tag=f"pt{i}", name=f"pt{i}")
                            nc.sync.dma_start(
                                pt_i, PT[i, :, :, rs].rearrange("c p r -> p c r")
                            )
                            pt.append(pt_i)
                        last = p2last.tile([128, 8, R2], mm_dtype)
                        for fc in range(8):
                            ps = psA.tile([128, R2], F32, tag="mm", name="ps2")
                            for i in range(3):
                                for kc in range(8):
                                    nc.tensor.matmul(
                                        ps,
                                        wh_sb[:, i * 8 + kc,
                                              fc * 128:(fc + 1) * 128],
                                        pt[i][:, kc, :],
                                        start=(i == 0 and kc == 0),
                                        stop=(i == 2 and kc == 7),
                                    )
                            nc.scalar.activation(
                                last[:, fc, :], ps, Act.Relu,
                                bias=bh_sb[:, fc:fc + 1],
                            )
                        nc.sync.dma_start(
                            dec[:, :, rs].rearrange("c p r -> p c r"), last
                        )

                        # alphas: partition-sum of last*P_i via ones-matmul
                        # (partition-redundant [128, R2])
                        aps = []
                        for i in range(3):
                            tmp = p2big.tile([128, 8, R2], mm_dtype,
                                             tag="tmp", name=f"tmp{i}",
                                             bufs=2)
                            eng = nc.gpsimd if i == 2 else nc.vector
                            eng.tensor_tensor(tmp, last, pt[i], AluOp.mult)
                            ap_i = psC.tile([128, R2], F32, tag="alpha",
                                            name=f"alpha{i}")
                            for fc in range(8):
                                nc.tensor.matmul(
                                    ap_i, ones_sb, tmp[:, fc, :],
                                    start=(fc == 0), stop=(fc == 7),
                                )
                            aps.append(ap_i)

                        # batched softmax over the 3 logits -> WS spill
                        asb = p2sm.tile([128, 3, R2], F32, tag="asb")
                        for i in range(3):
                            nc.scalar.copy(asb[:, i, :], aps[i])
                        ai = asb.rearrange("p i r -> p r i")
                        mx = p2sm.tile([128, R2], F32, tag="mx")
                        nc.vector.reduce_max(mx, ai, axis=mybir.AxisListType.X)
                        bshp = (128, 3, R2)
                        nc.vector.tensor_tensor(
                            asb, asb, mx[:, None, :].to_broadcast(bshp),
                            AluOp.subtract)
                        nc.scalar.activation(asb, asb, Act.Exp)
                        ssum = p2sm.tile([128, R2], F32, tag="ssum")
                        nc.vector.reduce_sum(ssum, ai, axis=mybir.AxisListType.X)
                        rcp = p2sm.tile([128, R2], F32, tag="rcp")
                        nc.vector.reciprocal(rcp, ssum)
                        wsr = p2sm.tile([128, 3, R2], mm_dtype, tag="wsr")
                        nc.vector.tensor_tensor(
                            wsr, asb, rcp[:, None, :].to_broadcast(bshp),
                            AluOp.mult)
                        nc.sync.dma_start(WS[:, :, rs], wsr)

            # ---- Decode: out_h = relu(dec @ wd1_h.T + bd1_h) @ wd2_h.T -----
            RF = 256
            NTF = ROWS // RF
            for h in range(2):
                with (
                    tc.tile_pool(name=f"fw{h}", bufs=1) as fw,
                    tc.tile_pool(name=f"fd{h}", bufs=3) as fd,
                    tc.tile_pool(name=f"fo{h}", bufs=2) as fo,
                    tc.tile_pool(name=f"fe{h}", bufs=2) as fe,
                    tc.tile_pool(name=f"psF{h}", bufs=4, space="PSUM") as psF,
                    tc.tile_pool(name=f"psG{h}", bufs=4, space="PSUM") as psG,
                ):
                    dcs = {}

                    def load_dc(rt, fd=fd):
                        rs = slice(rt * RF, (rt + 1) * RF)
                        t = fd.tile([128, 16, RF], mm_dtype, tag="dc",
                                    name="dc", bufs=2)
                        nc.sync.dma_start(
                            t[:, 8:16, :], dec[:, :, rs].rearrange("c p r -> p c r")
                        )
                        wf = fd.tile([128, 3, RF], mm_dtype, tag="wf",
                                     name="wf", bufs=2)
                        nc.sync.dma_start(wf, WS[:, :, rs])
                        dcs[rt] = (t, wf)

                    wd1_sb = fw.tile([128, 16, M], mm_dtype, tag="wd1")
                    wd2_sb = fw.tile([128, 8, OUT], mm_dtype, tag="wd2")
                    bd1_sb = fw.tile([128, 8], F32, tag="bd1")
                    bd2_sb = fw.tile([128, 8], F32, tag="bd2")
                    # per-chunk weight DMAs so the first matmuls start early
                    for kc in range(16):
                        nc.scalar.dma_start(
                            wd1_sb[:, kc, :],
                            wd1T.ap()[kc, :, h * M:(h + 1) * M],
                        )
                        if kc == 0:
                            load_dc(0)
                    for kc in range(8):
                        nc.scalar.dma_start(wd2_sb[:, kc, :],
                                            wd2T.ap()[h * 8 + kc])
                    nc.scalar.dma_start(bd1_sb, bd1.ap()[:, h * 8:(h + 1) * 8])
                    if h == 1:
                        nc.scalar.dma_start(bd2_sb, bd2.ap())

                    for rt in range(NTF):
                        rs = slice(rt * RF, (rt + 1) * RF)
                        if rt + 1 < NTF:
                            load_dc(rt + 1)
                        dc, wf = dcs.pop(rt)
                        pf = fd.tile([128, 24, RF], mm_dtype, tag="ptf",
                                     name="ptf", bufs=1)
                        nc.sync.dma_start(
                            pf, PT.rearrange("i c p r -> (i c) p r")[:, :, rs]
                            .rearrange("c p r -> p c r")
                        )
                        # ctx = sum_i ws_i * P_i, written into dc[:, 0:8]
                        shp = (128, 8, RF)
                        t2 = fo.tile([128, 8, RF], F32, tag="t2", name="t2")
                        t3 = fo.tile([128, 8, RF], F32, tag="t3", name="t3")
                        nc.vector.tensor_tensor(
                            dc[:, 0:8, :],
                            wf[:, 0, None, :].to_broadcast(shp),
                            pf[:, 0:8, :], AluOp.mult)
                        nc.vector.tensor_tensor(
                            t2, wf[:, 1, None, :].to_broadcast(shp),
                            pf[:, 8:16, :], AluOp.mult)
                        nc.gpsimd.tensor_tensor(
                            t3, wf[:, 2, None, :].to_broadcast(shp),
                            pf[:, 16:24, :], AluOp.mult)
                        nc.vector.tensor_tensor(
                            dc[:, 0:8, :], dc[:, 0:8, :], t2, AluOp.add)
                        nc.vector.tensor_tensor(
                            dc[:, 0:8, :], dc[:, 0:8, :], t3, AluOp.add)
                        o1 = fo.tile([128, 8, RF], mm_dtype)
                        for fc in range(8):
                            ps = psF.tile([128, RF], F32, tag="f1")
                            for kc in range(16):
                                nc.tensor.matmul(
                                    ps,
                                    wd1_sb[:, kc, fc * 128:(fc + 1) * 128],
                                    dc[:, kc, :],
                                    start=(kc == 0),
                                    stop=(kc == 15),
                                )
                            nc.scalar.activation(
                                o1[:, fc, :], ps, Act.Relu,
                                bias=bd1_sb[:, fc:fc + 1],
                            )
                        for oc in range(8):
                            ow = 128 if oc < 7 else OUT - 7 * 128
                            ps = psG.tile([128, RF], F32, tag="f2")
                            for kc in range(8):
                                nc.tensor.matmul(
                                    ps[:ow],
                                    wd2_sb[:, kc, oc * 128:oc * 128 + ow],
                                    o1[:, kc, :],
                                    start=(kc == 0),
                                    stop=(kc == 7),
                                )
                            ev = fe.tile([128, RF], F32)
                            if h == 1:
                                nc.vector.tensor_scalar_add(
                                    ev[:ow], ps[:ow], bd2_sb[:ow, oc:oc + 1]
                                )
                            else:
                                nc.vector.tensor_copy(ev[:ow], ps[:ow])
                            nc.sync.dma_start(
                                outH[h].ap()[oc * 128:oc * 128 + ow, rs],
                                ev[:ow],
                            )

    nc.finalize()
    return nc


def _prep_inputs(tube, w1_W, w1_b, w2_W, w2_b, w3_W, w3_b, wh_W, wh_b,
                 wd1_W, wd1_b, wd2_W, wd2_b):
    """Host-side reshape/transpose into the kernel's DRAM layouts."""
    f32 = np.float32

    def wT(w, kc):  # [F, K] -> [K, F] -> [kc, 128, F]
        w = np.asarray(w, f32)
        return np.ascontiguousarray(w.T).reshape(kc, 128, w.shape[0])

    def bmat(b, cc):  # [F] -> [128, cc]
        b = np.asarray(b, f32)
        if b.shape[0] < cc * 128:
            b = np.pad(b, (0, cc * 128 - b.shape[0]))
        return np.ascontiguousarray(b.reshape(cc, 128).T)

    shared = {
        "w1T": wT(w1_W, 4), "w2T": wT(w2_W, 4), "w3T": wT(w3_W, 4),
        "whT": wT(wh_W, 24), "wd1T": wT(wd1_W, 16), "wd2T": wT(wd2_W, 16),
        "b1": bmat(w1_b, 8), "b2": bmat(w2_b, 8), "b3": bmat(w3_b, 8),
        "bh": bmat(wh_b, 8), "bd1": bmat(wd1_b, 16), "bd2": bmat(wd2_b, 8),
    }
    tubeT = np.ascontiguousarray(np.asarray(tube, f32).T)  # [1536, B]
    in_maps = []
    for c in range(N_CORES):
        xTc = np.ascontiguousarray(
            tubeT[:, c * ROWS:(c + 1) * ROWS]
        ).reshape(12, 128, ROWS)
        in_maps.append({"xT": xTc, **shared})
    return in_maps


_NC_CACHE = {}


def run(inputs, mm_dtype=F32R, trace=False):
    key = (mm_dtype, )
    if key not in _NC_CACHE:
        _NC_CACHE[key] = build_nc(mm_dtype)
    nc = _NC_CACHE[key]
    in_maps = _prep_inputs(**inputs)
    res = run_bass_kernel_spmd(nc, in_maps, list(range(N_CORES)), trace=trace)
    out = np.empty((B, OUT), np.float32)
    for c in range(N_CORES):
        r = res.results[c]
        out[c * ROWS:(c + 1) * ROWS] = (r["out0"] + r["out1"]).T
    return out, res


def kernel(**inputs) -> np.ndarray:
    out, _ = run(inputs)
    return out



# revision 6
# speedup vs baseline: 1.1900x; 1.1900x over previous
"""Trainium2 Bass kernel for nn_ClassAtt (dense MLP + 3-way class attention).

Model (per row of tube [B, 1536]):
  x1,x2,x3 = tube split into 3x512
  P_i   = relu(x_i @ w_i.T + b_i)            [B, 1024]
  last  = relu(concat(P1,P2,P3) @ wh.T + bh) [B, 1024]
  a_i   = rowwise_dot(last, P_i); w = softmax(a)  [B, 3]
  ctx   = sum_i w_i * P_i                    [B, 1024]
  out   = relu(concat(ctx, last) @ wd1.T + bd1) @ wd2.T + bd2  [B, 1000]

Strategy (v3): pure data parallel over 8 NeuronCores (2048 rows each).
All matmuls run in bf16 (full PE rate, same as fp32r, half the DMA and
SBUF of fp32) with fp32 PSUM accumulation; end-to-end rel err vs the
fp32 reference is ~5e-3.  Activations live in transposed [feature, row]
layout so the contraction dim is always on SBUF partitions and biases
are per-partition scalars.

Two fused phases, one DRAM spill between them:
  Phase A (row tiles of 256): L1 (3x 512->1024) -> L2 (3072->1024) ->
    attention entirely in SBUF; spill dec = [ctx; last] (2048 feats,
    bf16).  Alphas: VectorE pre-reduces last*P_i over the 8 feature
    chunks, then a single fp32 ones-matmul per alpha does the
    partition reduction (keeps PE work minimal, all-fp32 accumulate).
  Phase B (row tiles of 512): out = relu(dec @ wd1.T + bd1) @ wd2.T
    + bd2, single pass; wd1 + wd2 both resident in bf16.

DMA plan (only sync/scalar have HWDGE rings; gpsimd is SWDGE):
  scalar ring: w1/w2/w3 + biases, then the xt stream (emitted after
    each tile's L1 so trigger waits can't stall activations).
  sync ring:   bh + wh chunks (so L2's FIFO-count wait covers only
    9.5MB and clears by ~20us), then dec half-tile loads in phase B.
  gpsimd:      wd1 prefetch during A, wd2 at the A->B transition,
    dec stores, out stores (last tile alternates sync to halve the
    final drain).
Phase-B pools open pbd first so dec loads land on the SBUF region
freed earliest by phase A (shorter WAR wait at the transition).
"""

import numpy as np
import ml_dtypes

import concourse.bass as bass
import concourse.mybir as mybir
import concourse.tile as tile
from concourse import bacc
from concourse.bass_utils import run_bass_kernel_spmd

F32 = mybir.dt.float32
F32R = mybir.dt.float32r
BF16 = mybir.dt.bfloat16

N_CORES = 8
B = 16384
ROWS = B // N_CORES  # rows per core
M = 1024             # hidden width
DEC_H = 2048
OUT = 1000

AluOp = mybir.AluOpType
Act = mybir.ActivationFunctionType


def build_nc():
    nc = bacc.Bacc(None, target_bir_lowering=False)

    # ---- DRAM I/O (per-core shapes) ----
    xT = nc.dram_tensor("xT", [12, 128, ROWS], BF16, kind="ExternalInput")
    wT = [
        nc.dram_tensor(f"w{i + 1}T", [4, 128, M], BF16, kind="ExternalInput")
        for i in range(3)
    ]
    whT = nc.dram_tensor("whT", [24, 128, M], BF16, kind="ExternalInput")
    wd1T = nc.dram_tensor("wd1T", [16, 128, DEC_H], BF16, kind="ExternalInput")
    wd2T = nc.dram_tensor("wd2T", [16, 128, OUT], BF16, kind="ExternalInput")
    bv = [
        nc.dram_tensor(f"b{i + 1}", [128, 8], F32, kind="ExternalInput")
        for i in range(3)
    ]
    bh = nc.dram_tensor("bh", [128, 8], F32, kind="ExternalInput")
    bd1 = nc.dram_tensor("bd1", [128, 16], F32, kind="ExternalInput")
    bd2 = nc.dram_tensor("bd2", [128, 8], F32, kind="ExternalInput")
    outT = nc.dram_tensor("outT", [OUT, ROWS], F32, kind="ExternalOutput")

    with tile.TileContext(nc) as tc:
        with tc.tile_pool(name="dram", bufs=1, space="DRAM") as dram:
            dec = dram.tile([16, 128, ROWS], BF16)  # [ctx; last], transposed

            # Outer pool: survives both phases (wd1 streams in during A).
            with tc.tile_pool(name="pw", bufs=1) as pw:
                wd1_sb = pw.tile([128, 16, DEC_H], BF16)
                bd1_sb = pw.tile([128, 16], F32, tag="bd1")
                bd2_sb = pw.tile([128, 8], F32, tag="bd2")
                ones_f32 = pw.tile([128, 128], F32, tag="ones_f32")

                # ================= Phase A =================
                R1 = 256
                NT1 = ROWS // R1
                with (
                    tc.tile_pool(name="paw", bufs=1) as paw,
                    tc.tile_pool(name="pax", bufs=2) as pax,
                    tc.tile_pool(name="pap", bufs=2) as pap,
                    tc.tile_pool(name="pad", bufs=2) as pad,
                    tc.tile_pool(name="pat", bufs=1) as pat,
                    tc.tile_pool(name="pas", bufs=1) as pas,
                    tc.tile_pool(name="psA", bufs=4, space="PSUM") as psA,
                    tc.tile_pool(name="psAl", bufs=3, space="PSUM") as psAl,
                ):
                    xts = {}

                    def load_xt(rt):
                        # 3 sub-DMAs (4 chunks each) for chunk-granular
                        # FIFO-count waits on the scalar ring.
                        rs = slice(rt * R1, (rt + 1) * R1)
                        t = pax.tile([128, 12, R1], BF16, tag="xt", name="xt")
                        for g in range(3):
                            nc.scalar.dma_start(
                                t[:, 4 * g:4 * g + 4, :],
                                xT.ap()[4 * g:4 * g + 4, :, rs]
                                .rearrange("c p r -> p c r"),
                            )
                        xts[rt] = t

                    # scalar ring: w1/b1 first (L1 tile 0 cannot start
                    # without them), then xt0/xt1, then w2/w3.
                    w_sb = []
                    b_sb = []
                    for i in range(3):
                        w = paw.tile([128, 4, M], BF16, tag=f"w{i}",
                                     name=f"w{i}")
                        nc.scalar.dma_start(w, wT[i].ap().rearrange("c p f -> p c f"))
                        b = paw.tile([128, 8], F32, tag=f"b{i}", name=f"b{i}")
                        nc.scalar.dma_start(b, bv[i].ap())
                        w_sb.append(w)
                        b_sb.append(b)
                        if i == 0:
                            load_xt(0)
                        if i == 1:
                            load_xt(1)
                    # sync ring: bh + wh only -> L2's wait clears early.
                    bh_sb = paw.tile([128, 8], F32, tag="bh", name="bh")
                    nc.sync.dma_start(bh_sb, bh.ap())
                    wh_sb = paw.tile([128, 24, M], BF16, tag="wh", name="wh")
                    for c in range(24):
                        nc.sync.dma_start(wh_sb[:, c, :], whT.ap()[c])
                    # gpsimd (SWDGE): bulk wd1 prefetch for phase B.
                    for kc in range(16):
                        nc.gpsimd.dma_start(wd1_sb[:, kc, :], wd1T.ap()[kc])
                    nc.gpsimd.dma_start(bd1_sb, bd1.ap())
                    nc.gpsimd.dma_start(bd2_sb, bd2.ap())
                    nc.any.memset(ones_f32, 1.0)

                    for rt in range(NT1):
                        rs = slice(rt * R1, (rt + 1) * R1)
                        xt = xts.pop(rt)

                        # ---- L1: P_i = relu(x_i @ w_i.T + b_i) ----
                        pt = []
                        for i in range(3):
                            p_i = pap.tile([128, 8, R1], BF16, tag=f"p{i}",
                                           name=f"p{i}")
                            for fc in range(8):
                                ps = psA.tile([128, R1], F32, tag="mm",
                                              name="ps1")
                                for kc in range(4):
                                    nc.tensor.matmul(
                                        ps,
                                        w_sb[i][:, kc, fc * 128:(fc + 1) * 128],
                                        xt[:, i * 4 + kc, :],
                                        start=(kc == 0),
                                        stop=(kc == 3),
                                    )
                                nc.scalar.activation(
                                    p_i[:, fc, :], ps, Act.Relu,
                                    bias=b_sb[i][:, fc:fc + 1],
                                )
                            pt.append(p_i)

                        # xt prefetch AFTER L1 emission: the trigger's WAR
                        # wait (xt buf reuse) then can't stall activations.
                        if rt + 2 < NT1:
                            load_xt(rt + 2)

                        # ---- L2: last = relu(hid1 @ wh.T + bh) ----
                        dec_sb = pad.tile([128, 16, R1], BF16, tag="dec",
                                          name="dec")
                        last = dec_sb[:, 8:16, :]
                        for fc in range(8):
                            ps = psA.tile([128, R1], F32, tag="mm", name="ps2")
                            for i in range(3):
                                for kc in range(8):
                                    nc.tensor.matmul(
                                        ps,
                                        wh_sb[:, i * 8 + kc,
                                              fc * 128:(fc + 1) * 128],
                                        pt[i][:, kc, :],
                                        start=(i == 0 and kc == 0),
                                        stop=(i == 2 and kc == 7),
                                    )
                            nc.scalar.activation(
                                last[:, fc, :], ps, Act.Relu,
                                bias=bh_sb[:, fc:fc + 1],
                            )

                        # ---- attention ----
                        # alpha_i = sum_p sum_c (last*P_i)[p,c,r]:
                        # VectorE reduces over chunks (fp32), one fp32
                        # ones-matmul reduces over partitions.
                        aps = []
                        for i in range(3):
                            tmp = pat.tile([128, 8, R1], BF16, tag="tmp",
                                           name=f"tmp{i}")
                            nc.vector.tensor_tensor(tmp, last, pt[i],
                                                    AluOp.mult)
                            s_i = pas.tile([128, R1], F32, tag="s",
                                           name=f"s{i}")
                            nc.vector.reduce_sum(
                                s_i, tmp.rearrange("p c r -> p r c"),
                                axis=mybir.AxisListType.X)
                            ap_i = psAl.tile([128, R1], F32, tag="alpha",
                                             name=f"alpha{i}")
                            nc.tensor.matmul(ap_i, ones_f32, s_i,
                                             start=True, stop=True)
                            aps.append(ap_i)

                        # softmax over the 3 logits (fp32)
                        asb = pas.tile([128, 3, R1], F32, tag="asb")
                        for i in range(3):
                            nc.scalar.copy(asb[:, i, :], aps[i])
                        ai = asb.rearrange("p i r -> p r i")
                        mx = pas.tile([128, R1], F32, tag="mx")
                        nc.vector.reduce_max(mx, ai, axis=mybir.AxisListType.X)
                        bshp = (128, 3, R1)
                        nc.vector.tensor_tensor(
                            asb, asb, mx[:, None, :].to_broadcast(bshp),
                            AluOp.subtract)
                        nc.scalar.activation(asb, asb, Act.Exp)
                        ssum = pas.tile([128, R1], F32, tag="ssum")
                        nc.vector.reduce_sum(ssum, ai, axis=mybir.AxisListType.X)
                        rcp = pas.tile([128, R1], F32, tag="rcp")
                        nc.vector.reciprocal(rcp, ssum)
                        wsr = pas.tile([128, 3, R1], BF16, tag="wsr")
                        nc.vector.tensor_tensor(
                            wsr, asb, rcp[:, None, :].to_broadcast(bshp),
                            AluOp.mult)

                        # ctx = sum_i ws_i * P_i -> dec_sb[:, 0:8]
                        shp = (128, 8, R1)
                        ctx = dec_sb[:, 0:8, :]
                        nc.vector.tensor_tensor(
                            ctx, wsr[:, 0, None, :].to_broadcast(shp),
                            pt[0], AluOp.mult)
                        t2 = pat.tile([128, 8, R1], BF16, tag="tmp", name="t2")
                        nc.vector.tensor_tensor(
                            t2, wsr[:, 1, None, :].to_broadcast(shp),
                            pt[1], AluOp.mult)
                        nc.vector.tensor_tensor(ctx, ctx, t2, AluOp.add)
                        t3 = pat.tile([128, 8, R1], BF16, tag="tmp", name="t3")
                        nc.vector.tensor_tensor(
                            t3, wsr[:, 2, None, :].to_broadcast(shp),
                            pt[2], AluOp.mult)
                        nc.vector.tensor_tensor(ctx, ctx, t3, AluOp.add)

                        nc.gpsimd.dma_start(
                            dec[:, :, rs].rearrange("c p r -> p c r"), dec_sb
                        )

                # ================= Phase B =================
                RF = 512
                NTF = ROWS // RF
                with (
                    # pbd first: lands on the SBUF region freed earliest.
                    tc.tile_pool(name="pbd", bufs=2) as pbd,
                    tc.tile_pool(name="pbo", bufs=2) as pbo,
                    tc.tile_pool(name="pbe", bufs=3) as pbe,
                    tc.tile_pool(name="pbw", bufs=1) as pbw,
                    tc.tile_pool(name="psD", bufs=3, space="PSUM") as psD,
                    tc.tile_pool(name="psE", bufs=3, space="PSUM") as psE,
                ):
                    dcs = {}

                    def load_dc(rt):
                        # two half-tile DMAs -> D1 kc 0-7 can start while
                        # the second half is still in flight.
                        rs = slice(rt * RF, (rt + 1) * RF)
                        ta = pbd.tile([128, 8, RF], BF16, tag="dca",
                                      name="dca")
                        tb = pbd.tile([128, 8, RF], BF16, tag="dcb",
                                      name="dcb")
                        nc.sync.dma_start(
                            ta, dec[0:8, :, rs].rearrange("c p r -> p c r")
                        )
                        nc.sync.dma_start(
                            tb, dec[8:16, :, rs].rearrange("c p r -> p c r")
                        )
                        dcs[rt] = (ta, tb)

                    load_dc(0)
                    # wd2 on gpsimd: behind the dec stores in that FIFO,
                    # ready well before the first D2 needs it.
                    wd2_sb = pbw.tile([128, 16, OUT], BF16, tag="wd2")
                    for kc in range(16):
                        nc.gpsimd.dma_start(wd2_sb[:, kc, :], wd2T.ap()[kc])

                    for rt in range(NTF):
                        rs = slice(rt * RF, (rt + 1) * RF)
                        if rt + 1 < NTF:
                            load_dc(rt + 1)
                        dca, dcb = dcs.pop(rt)

                        o1 = pbo.tile([128, 16, RF], BF16, tag="o1",
                                      name="o1")
                        for fc in range(16):
                            ps = psD.tile([128, RF], F32, tag="d1")
                            for kc in range(16):
                                src = dca if kc < 8 else dcb
                                nc.tensor.matmul(
                                    ps,
                                    wd1_sb[:, kc, fc * 128:(fc + 1) * 128],
                                    src[:, kc % 8, :],
                                    start=(kc == 0),
                                    stop=(kc == 15),
                                )
                            nc.scalar.activation(
                                o1[:, fc, :], ps, Act.Relu,
                                bias=bd1_sb[:, fc:fc + 1],
                            )
                        for oc in range(8):
                            ow = 128 if oc < 7 else OUT - 7 * 128
                            ps = psE.tile([128, RF], F32, tag="d2")
                            for kc in range(16):
                                nc.tensor.matmul(
                                    ps[:ow],
                                    wd2_sb[:, kc, oc * 128:oc * 128 + ow],
                                    o1[:, kc, :],
                                    start=(kc == 0),
                                    stop=(kc == 15),
                                )
                            ev = pbe.tile([128, RF], F32, tag="ev")
                            nc.vector.tensor_scalar_add(
                                ev[:ow], ps[:ow], bd2_sb[:ow, oc:oc + 1]
                            )
                            # last tile: split the drain across two queues
                            eng = (nc.sync if (rt == NTF - 1 and oc % 2)
                                   else nc.gpsimd)
                            eng.dma_start(
                                outT.ap()[oc * 128:oc * 128 + ow, rs],
                                ev[:ow],
                            )

    nc.finalize()
    return nc


def _prep_inputs(tube, w1_W, w1_b, w2_W, w2_b, w3_W, w3_b, wh_W, wh_b,
                 wd1_W, wd1_b, wd2_W, wd2_b):
    """Host-side reshape/transpose into the kernel's DRAM layouts."""
    f32 = np.float32
    bf16 = ml_dtypes.bfloat16

    def wT(w, kc):  # [F, K] -> [K, F] -> [kc, 128, F], bf16
        w = np.asarray(w, f32)
        return np.ascontiguousarray(w.T.astype(bf16)).reshape(
            kc, 128, w.shape[0])

    def bmat(b, cc):  # [F] -> [128, cc]
        b = np.asarray(b, f32)
        if b.shape[0] < cc * 128:
            b = np.pad(b, (0, cc * 128 - b.shape[0]))
        return np.ascontiguousarray(b.reshape(cc, 128).T)

    shared = {
        "w1T": wT(w1_W, 4), "w2T": wT(w2_W, 4), "w3T": wT(w3_W, 4),
        "whT": wT(wh_W, 24), "wd1T": wT(wd1_W, 16), "wd2T": wT(wd2_W, 16),
        "b1": bmat(w1_b, 8), "b2": bmat(w2_b, 8), "b3": bmat(w3_b, 8),
        "bh": bmat(wh_b, 8), "bd1": bmat(wd1_b, 16), "bd2": bmat(wd2_b, 8),
    }
    tubeT = np.ascontiguousarray(np.asarray(tube, f32).T.astype(bf16))
    in_maps = []
    for c in range(N_CORES):
        xTc = np.ascontiguousarray(
            tubeT[:, c * ROWS:(c + 1) * ROWS]
        ).reshape(12, 128, ROWS)
        in_maps.append({"xT": xTc, **shared})
    return in_maps


_NC_CACHE = {}


def run(inputs, mm_dtype=None, trace=False):
    # mm_dtype kept for test.py compat; the kernel is bf16-only now.
    if "nc" not in _NC_CACHE:
        _NC_CACHE["nc"] = build_nc()
    nc = _NC_CACHE["nc"]
    in_maps = _prep_inputs(**inputs)
    res = run_bass_kernel_spmd(nc, in_maps, list(range(N_CORES)), trace=trace)
    out = np.empty((B, OUT), np.float32)
    for c in range(N_CORES):
        out[c * ROWS:(c + 1) * ROWS] = res.results[c]["outT"].T
    return out, res


def kernel(**inputs) -> np.ndarray:
    out, _ = run(inputs)
    return out


# revision 9
# speedup vs baseline: 1.4424x; 1.2120x over previous
"""Trainium2 Bass kernel for nn_ClassAtt (dense MLP + 3-way class attention).

Model (per row of tube [B, 1536]):
  x1,x2,x3 = tube split into 3x512
  P_i   = relu(x_i @ w_i.T + b_i)            [B, 1024]
  last  = relu(concat(P1,P2,P3) @ wh.T + bh) [B, 1024]
  a_i   = rowwise_dot(last, P_i); w = softmax(a)  [B, 3]
  ctx   = sum_i w_i * P_i                    [B, 1024]
  out   = relu(concat(ctx, last) @ wd1.T + bd1) @ wd2.T + bd2  [B, 1000]

Strategy (v3): pure data parallel over 8 NeuronCores (2048 rows each).
All matmuls run in bf16 (full PE rate, same as fp32r, half the DMA and
SBUF of fp32) with fp32 PSUM accumulation; end-to-end rel err vs the
fp32 reference is ~5e-3.  Activations live in transposed [feature, row]
layout so the contraction dim is always on SBUF partitions and biases
are per-partition scalars.

Two fused phases, one DRAM spill between them:
  Phase A (row tiles of 256): L1 (3x 512->1024) -> L2 (3072->1024) ->
    attention entirely in SBUF; spill dec = [ctx; last] (2048 feats,
    bf16).  Alphas: VectorE pre-reduces last*P_i over the 8 feature
    chunks, then a single fp32 ones-matmul per alpha does the
    partition reduction (keeps PE work minimal, all-fp32 accumulate).
  Phase B (row tiles of 512): out = relu(dec @ wd1.T + bd1) @ wd2.T
    + bd2, single pass; wd1 + wd2 both resident in bf16.

DMA plan (only sync/scalar have HWDGE rings; gpsimd is SWDGE):
  scalar ring: w1/w2/w3 + biases, then the xt stream (emitted after
    each tile's L1 so trigger waits can't stall activations).
  sync ring:   bh + wh chunks (so L2's FIFO-count wait covers only
    9.5MB and clears by ~20us), then dec half-tile loads in phase B.
  gpsimd:      wd1 prefetch during A, wd2 at the A->B transition,
    dec stores, out stores (last tile alternates sync to halve the
    final drain).
Phase-B pools open pbd first so dec loads land on the SBUF region
freed earliest by phase A (shorter WAR wait at the transition).
"""

import numpy as np
import ml_dtypes

import concourse.bass as bass
import concourse.mybir as mybir
import concourse.tile as tile
from concourse import bacc
from concourse.bass_utils import run_bass_kernel_spmd

F32 = mybir.dt.float32
F32R = mybir.dt.float32r
BF16 = mybir.dt.bfloat16

N_CORES = 8
B = 16384
ROWS = B // N_CORES  # rows per core
M = 1024             # hidden width
DEC_H = 2048
OUT = 1000

AluOp = mybir.AluOpType
Act = mybir.ActivationFunctionType


def build_nc():
    nc = bacc.Bacc(None, target_bir_lowering=False)

    # ---- DRAM I/O (per-core shapes) ----
    xT = nc.dram_tensor("xT", [12, 128, ROWS], BF16, kind="ExternalInput")
    wT = [
        nc.dram_tensor(f"w{i + 1}T", [4, 128, M], BF16, kind="ExternalInput")
        for i in range(3)
    ]
    whT = nc.dram_tensor("whT", [24, 128, M], BF16, kind="ExternalInput")
    wd1T = nc.dram_tensor("wd1T", [16, 128, DEC_H], BF16, kind="ExternalInput")
    wd2T = nc.dram_tensor("wd2T", [16, 128, OUT], BF16, kind="ExternalInput")
    bv = [
        nc.dram_tensor(f"b{i + 1}", [128, 8], F32, kind="ExternalInput")
        for i in range(3)
    ]
    bh = nc.dram_tensor("bh", [128, 8], F32, kind="ExternalInput")
    bd1 = nc.dram_tensor("bd1", [128, 16], F32, kind="ExternalInput")
    bd2 = nc.dram_tensor("bd2", [128, 8], F32, kind="ExternalInput")
    outT = nc.dram_tensor("outT", [OUT, ROWS], F32, kind="ExternalOutput")

    with tile.TileContext(nc) as tc:
        with tc.tile_pool(name="dram", bufs=1, space="DRAM") as dram:
            dec = dram.tile([16, 128, ROWS], BF16)  # [ctx; last], transposed

            # Outer pool: survives both phases (wd1 streams in during A).
            with tc.tile_pool(name="pw", bufs=1) as pw:
                wd1_sb = pw.tile([128, 16, DEC_H], BF16)
                bd1_sb = pw.tile([128, 16], F32, tag="bd1")
                bd2_sb = pw.tile([128, 8], F32, tag="bd2")
                ones_f32 = pw.tile([128, 128], F32, tag="ones_f32")
                ones_sb = pw.tile([128, 128], BF16, tag="ones")

                # ================= Phase A =================
                R1 = 256
                NT1 = ROWS // R1
                with (
                    tc.tile_pool(name="paw", bufs=1) as paw,
                    tc.tile_pool(name="pax", bufs=2) as pax,
                    tc.tile_pool(name="pap", bufs=2) as pap,
                    tc.tile_pool(name="pad", bufs=2) as pad,
                    tc.tile_pool(name="pat", bufs=1) as pat,
                    tc.tile_pool(name="pas", bufs=1) as pas,
                    tc.tile_pool(name="psA", bufs=4, space="PSUM") as psA,
                    tc.tile_pool(name="psAl", bufs=3, space="PSUM") as psAl,
                ):
                    xts = {}

                    def load_xt(rt):
                        # sync ring; 3 sub-DMAs (4 chunks each) for
                        # chunk-granular FIFO-count waits.
                        rs = slice(rt * R1, (rt + 1) * R1)
                        t = pax.tile([128, 12, R1], BF16, tag="xt", name="xt")
                        for g in range(3):
                            nc.sync.dma_start(
                                t[:, 4 * g:4 * g + 4, :],
                                xT.ap()[4 * g:4 * g + 4, :, rs]
                                .rearrange("c p r -> p c r"),
                            )
                        xts[rt] = t

                    # scalar ring (fast): w1/b1 first, then w2/w3, bh, wh.
                    # Nothing else shares this ring, so L2's FIFO-count
                    # wait covers only these 9.5MB.
                    w_sb = []
                    b_sb = []
                    for i in range(3):
                        w = paw.tile([128, 4, M], BF16, tag=f"w{i}",
                                     name=f"w{i}")
                        nc.scalar.dma_start(w, wT[i].ap().rearrange("c p f -> p c f"))
                        b = paw.tile([128, 8], F32, tag=f"b{i}", name=f"b{i}")
                        nc.scalar.dma_start(b, bv[i].ap())
                        w_sb.append(w)
                        b_sb.append(b)
                        if i == 0:
                            load_xt(0)
                        if i == 1:
                            load_xt(1)
                    bh_sb = paw.tile([128, 8], F32, tag="bh", name="bh")
                    nc.scalar.dma_start(bh_sb, bh.ap())
                    wh_sb = paw.tile([128, 24, M], BF16, tag="wh", name="wh")
                    for c in range(24):
                        nc.scalar.dma_start(wh_sb[:, c, :], whT.ap()[c])
                    # gpsimd (SWDGE): bulk wd1 prefetch for phase B.
                    for kc in range(16):
                        nc.gpsimd.dma_start(wd1_sb[:, kc, :], wd1T.ap()[kc])
                    nc.gpsimd.dma_start(bd1_sb, bd1.ap())
                    nc.gpsimd.dma_start(bd2_sb, bd2.ap())
                    nc.any.memset(ones_f32, 1.0)
                    nc.vector.tensor_copy(ones_sb, ones_f32)

                    for rt in range(NT1):
                        rs = slice(rt * R1, (rt + 1) * R1)
                        xt = xts.pop(rt)

                        # ---- L1: P_i = relu(x_i @ w_i.T + b_i) ----
                        pt = []
                        for i in range(3):
                            p_i = pap.tile([128, 8, R1], BF16, tag=f"p{i}",
                                           name=f"p{i}")
                            for fc in range(8):
                                ps = psA.tile([128, R1], F32, tag="mm",
                                              name="ps1")
                                for kc in range(4):
                                    nc.tensor.matmul(
                                        ps,
                                        w_sb[i][:, kc, fc * 128:(fc + 1) * 128],
                                        xt[:, i * 4 + kc, :],
                                        start=(kc == 0),
                                        stop=(kc == 3),
                                    )
                                nc.scalar.activation(
                                    p_i[:, fc, :], ps, Act.Relu,
                                    bias=b_sb[i][:, fc:fc + 1],
                                )
                            pt.append(p_i)

                        # xt prefetch AFTER L1 emission: the trigger's WAR
                        # wait (xt buf reuse) then can't stall activations.
                        if rt + 2 < NT1:
                            load_xt(rt + 2)

                        # ---- L2: last = relu(hid1 @ wh.T + bh) ----
                        dec_sb = pad.tile([128, 16, R1], BF16, tag="dec",
                                          name="dec")
                        last = dec_sb[:, 8:16, :]
                        for fc in range(8):
                            ps = psA.tile([128, R1], F32, tag="mm", name="ps2")
                            for i in range(3):
                                for kc in range(8):
                                    nc.tensor.matmul(
                                        ps,
                                        wh_sb[:, i * 8 + kc,
                                              fc * 128:(fc + 1) * 128],
                                        pt[i][:, kc, :],
                                        start=(i == 0 and kc == 0),
                                        stop=(i == 2 and kc == 7),
                                    )
                            nc.scalar.activation(
                                last[:, fc, :], ps, Act.Relu,
                                bias=bh_sb[:, fc:fc + 1],
                            )

                        # ---- attention: alphas via bf16 ones-matmul ----
                        aps = []
                        for i in range(3):
                            tmp = pat.tile([128, 8, R1], BF16, tag="tmp",
                                           name=f"tmp{i}")
                            nc.vector.tensor_tensor(tmp, last, pt[i],
                                                    AluOp.mult)
                            ap_i = psAl.tile([128, R1], F32, tag="alpha",
                                             name=f"alpha{i}")
                            for fc in range(8):
                                nc.tensor.matmul(
                                    ap_i, ones_sb, tmp[:, fc, :],
                                    start=(fc == 0), stop=(fc == 7),
                                )
                            aps.append(ap_i)

                        # softmax over the 3 logits (fp32)
                        asb = pas.tile([128, 3, R1], F32, tag="asb")
                        for i in range(3):
                            nc.scalar.copy(asb[:, i, :], aps[i])
                        ai = asb.rearrange("p i r -> p r i")
                        mx = pas.tile([128, R1], F32, tag="mx")
                        nc.vector.reduce_max(mx, ai, axis=mybir.AxisListType.X)
                        bshp = (128, 3, R1)
                        nc.vector.tensor_tensor(
                            asb, asb, mx[:, None, :].to_broadcast(bshp),
                            AluOp.subtract)
                        nc.scalar.activation(asb, asb, Act.Exp)
                        ssum = pas.tile([128, R1], F32, tag="ssum")
                        nc.vector.reduce_sum(ssum, ai, axis=mybir.AxisListType.X)
                        rcp = pas.tile([128, R1], F32, tag="rcp")
                        nc.vector.reciprocal(rcp, ssum)
                        wsr = pas.tile([128, 3, R1], BF16, tag="wsr")
                        nc.vector.tensor_tensor(
                            wsr, asb, rcp[:, None, :].to_broadcast(bshp),
                            AluOp.mult)

                        # ctx = sum_i ws_i * P_i -> dec_sb[:, 0:8]
                        shp = (128, 8, R1)
                        ctx = dec_sb[:, 0:8, :]
                        nc.vector.tensor_tensor(
                            ctx, wsr[:, 0, None, :].to_broadcast(shp),
                            pt[0], AluOp.mult)
                        t2 = pat.tile([128, 8, R1], BF16, tag="tmp", name="t2")
                        nc.vector.tensor_tensor(
                            t2, wsr[:, 1, None, :].to_broadcast(shp),
                            pt[1], AluOp.mult)
                        nc.vector.tensor_tensor(ctx, ctx, t2, AluOp.add)
                        t3 = pat.tile([128, 8, R1], BF16, tag="tmp", name="t3")
                        nc.vector.tensor_tensor(
                            t3, wsr[:, 2, None, :].to_broadcast(shp),
                            pt[2], AluOp.mult)
                        nc.vector.tensor_tensor(ctx, ctx, t3, AluOp.add)

                        nc.gpsimd.dma_start(
                            dec[:, :, rs].rearrange("c p r -> p c r"), dec_sb
                        )

                # ================= Phase B =================
                RF = 512
                NTF = ROWS // RF
                with (
                    # pbd first: lands on the SBUF region freed earliest.
                    tc.tile_pool(name="pbd", bufs=2) as pbd,
                    tc.tile_pool(name="pbo", bufs=2) as pbo,
                    tc.tile_pool(name="pbe", bufs=3) as pbe,
                    tc.tile_pool(name="pbw", bufs=1) as pbw,
                    tc.tile_pool(name="psD", bufs=3, space="PSUM") as psD,
                    tc.tile_pool(name="psE", bufs=3, space="PSUM") as psE,
                ):
                    dcs = {}

                    def load_dc(rt):
                        # two half-tile DMAs -> D1 kc 0-7 can start while
                        # the second half is still in flight.
                        rs = slice(rt * RF, (rt + 1) * RF)
                        ta = pbd.tile([128, 8, RF], BF16, tag="dca",
                                      name="dca")
                        tb = pbd.tile([128, 8, RF], BF16, tag="dcb",
                                      name="dcb")
                        nc.sync.dma_start(
                            ta, dec[0:8, :, rs].rearrange("c p r -> p c r")
                        )
                        nc.sync.dma_start(
                            tb, dec[8:16, :, rs].rearrange("c p r -> p c r")
                        )
                        dcs[rt] = (ta, tb)

                    load_dc(0)
                    # wd2 on gpsimd: behind the dec stores in that FIFO,
                    # ready well before the first D2 needs it.
                    wd2_sb = pbw.tile([128, 16, OUT], BF16, tag="wd2")
                    for kc in range(16):
                        nc.gpsimd.dma_start(wd2_sb[:, kc, :], wd2T.ap()[kc])

                    for rt in range(NTF):
                        rs = slice(rt * RF, (rt + 1) * RF)
                        if rt + 1 < NTF:
                            load_dc(rt + 1)
                        dca, dcb = dcs.pop(rt)

                        o1 = pbo.tile([128, 16, RF], BF16, tag="o1",
                                      name="o1")
                        for fc in range(16):
                            ps = psD.tile([128, RF], F32, tag="d1")
                            for kc in range(16):
                                src = dca if kc < 8 else dcb
                                nc.tensor.matmul(
                                    ps,
                                    wd1_sb[:, kc, fc * 128:(fc + 1) * 128],
                                    src[:, kc % 8, :],
                                    start=(kc == 0),
                                    stop=(kc == 15),
                                )
                            nc.scalar.activation(
                                o1[:, fc, :], ps, Act.Relu,
                                bias=bd1_sb[:, fc:fc + 1],
                            )
                        for oc in range(8):
                            ow = 128 if oc < 7 else OUT - 7 * 128
                            ps = psE.tile([128, RF], F32, tag="d2")
                            for kc in range(16):
                                nc.tensor.matmul(
                                    ps[:ow],
                                    wd2_sb[:, kc, oc * 128:oc * 128 + ow],
                                    o1[:, kc, :],
                                    start=(kc == 0),
                                    stop=(kc == 15),
                                )
                            ev = pbe.tile([128, RF], F32, tag="ev")
                            nc.vector.tensor_scalar_add(
                                ev[:ow], ps[:ow], bd2_sb[:ow, oc:oc + 1]
                            )
                            # last tile: split the drain across two queues
                            eng = (nc.sync if (rt == NTF - 1 and oc % 2)
                                   else nc.gpsimd)
                            eng.dma_start(
                                outT.ap()[oc * 128:oc * 128 + ow, rs],
                                ev[:ow],
                            )

    nc.finalize()
    return nc


def _prep_inputs(tube, w1_W, w1_b, w2_W, w2_b, w3_W, w3_b, wh_W, wh_b,
                 wd1_W, wd1_b, wd2_W, wd2_b):
    """Host-side reshape/transpose into the kernel's DRAM layouts."""
    f32 = np.float32
    bf16 = ml_dtypes.bfloat16

    def wT(w, kc):  # [F, K] -> [K, F] -> [kc, 128, F], bf16
        w = np.asarray(w, f32)
        return np.ascontiguousarray(w.T.astype(bf16)).reshape(
            kc, 128, w.shape[0])

    def bmat(b, cc):  # [F] -> [128, cc]
        b = np.asarray(b, f32)
        if b.shape[0] < cc * 128:
            b = np.pad(b, (0, cc * 128 - b.shape[0]))
        return np.ascontiguousarray(b.reshape(cc, 128).T)

    shared = {
        "w1T": wT(w1_W, 4), "w2T": wT(w2_W, 4), "w3T": wT(w3_W, 4),
        "whT": wT(wh_W, 24), "wd1T": wT(wd1_W, 16), "wd2T": wT(wd2_W, 16),
        "b1": bmat(w1_b, 8), "b2": bmat(w2_b, 8), "b3": bmat(w3_b, 8),
        "bh": bmat(wh_b, 8), "bd1": bmat(wd1_b, 16), "bd2": bmat(wd2_b, 8),
    }
    tubeT = np.ascontiguousarray(np.asarray(tube, f32).T.astype(bf16))
    in_maps = []
    for c in range(N_CORES):
        xTc = np.ascontiguousarray(
            tubeT[:, c * ROWS:(c + 1) * ROWS]
        ).reshape(12, 128, ROWS)
        in_maps.append({"xT": xTc, **shared})
    return in_maps


_NC_CACHE = {}


def run(inputs, mm_dtype=None, trace=False):
    # mm_dtype kept for test.py compat; the kernel is bf16-only now.
    if "nc" not in _NC_CACHE:
        _NC_CACHE["nc"] = build_nc()
    nc = _NC_CACHE["nc"]
    in_maps = _prep_inputs(**inputs)
    res = run_bass_kernel_spmd(nc, in_maps, list(range(N_CORES)), trace=trace)
    out = np.empty((B, OUT), np.float32)
    for c in range(N_CORES):
        out[c * ROWS:(c + 1) * ROWS] = res.results[c]["outT"].T
    return out, res


def kernel(**inputs) -> np.ndarray:
    out, _ = run(inputs)
    return out


# revision 10
# speedup vs baseline: 1.4635x; 1.0146x over previous
"""Trainium2 Bass kernel for nn_ClassAtt (dense MLP + 3-way class attention).

Model (per row of tube [B, 1536]):
  x1,x2,x3 = tube split into 3x512
  P_i   = relu(x_i @ w_i.T + b_i)            [B, 1024]
  last  = relu(concat(P1,P2,P3) @ wh.T + bh) [B, 1024]
  a_i   = rowwise_dot(last, P_i); w = softmax(a)  [B, 3]
  ctx   = sum_i w_i * P_i                    [B, 1024]
  out   = relu(concat(ctx, last) @ wd1.T + bd1) @ wd2.T + bd2  [B, 1000]

Strategy (v5): pure data parallel over 8 NeuronCores (2048 rows each).
All matmuls in bf16 (full PE rate, half the DMA/SBUF of fp32) with fp32
PSUM accumulation; rel err vs fp32 reference ~5.5e-3.  Activations in
transposed [feature, row] layout: contraction on SBUF partitions,
biases per-partition scalars.

Two fused phases, one DRAM spill (dec=[ctx;last]) between them; row
tiles of 256 in both.  A 10-matmul fp32 warmup at t=0 keeps the PE HAM
clock-gate warm through the initial weight load.

DMA: SDMA engines round-robin *packet slots* across active queues, so
per-queue bandwidth ~ packet size.  Every DRAM tensor is therefore
host-prearranged into its exact SBUF layout (partition-major) so each
transfer is a single DMA with multi-KB per-partition contiguity:
  xT [8,128,12,256] (one 6KB/part DMA per row tile, sync ring)
  w1/w2/w3 [128,4,1024], wh [128,24,1024] (scalar ring; wh in 4 groups
    so L2 tile 0 starts as chunks land)
  wd1 [128,16,2048] (gpsimd SWDGE, 3 chunks per row tile from tile 1 —
    off the scalar ring so L2's FIFO-count wait never covers it, and
    spread so it doesn't steal packet slots from wh)
  dec spill [8,128,16,256] A-tile-major (8KB/part contiguous both ways)
  out writes alternate sync/gpsimd to halve the final drain.
Phase-B pools open pbd first so dec loads land on the SBUF region
freed earliest by phase A (shortest WAR wait at the transition).
"""

import numpy as np
import ml_dtypes

import concourse.bass as bass
import concourse.mybir as mybir
import concourse.tile as tile
from concourse import bacc
from concourse.bass_utils import run_bass_kernel_spmd

F32 = mybir.dt.float32
F32R = mybir.dt.float32r
BF16 = mybir.dt.bfloat16

N_CORES = 8
B = 16384
ROWS = B // N_CORES  # rows per core
M = 1024             # hidden width
DEC_H = 2048
OUT = 1000
R1 = 256
NT1 = ROWS // R1

AluOp = mybir.AluOpType
Act = mybir.ActivationFunctionType


def build_nc():
    nc = bacc.Bacc(None, target_bir_lowering=False)

    # ---- DRAM I/O (per-core shapes, partition-major SBUF layouts) ----
    xT = nc.dram_tensor("xT", [NT1, 128, 12, R1], BF16, kind="ExternalInput")
    wT = [
        nc.dram_tensor(f"w{i + 1}T", [128, 4, M], BF16, kind="ExternalInput")
        for i in range(3)
    ]
    whT = nc.dram_tensor("whT", [128, 24, M], BF16, kind="ExternalInput")
    wd1T = nc.dram_tensor("wd1T", [128, 16, DEC_H], BF16, kind="ExternalInput")
    wd2T = nc.dram_tensor("wd2T", [128, 16, OUT], BF16, kind="ExternalInput")
    bv = [
        nc.dram_tensor(f"b{i + 1}", [128, 8], F32, kind="ExternalInput")
        for i in range(3)
    ]
    bh = nc.dram_tensor("bh", [128, 8], F32, kind="ExternalInput")
    bd1 = nc.dram_tensor("bd1", [128, 16], F32, kind="ExternalInput")
    bd2 = nc.dram_tensor("bd2", [128, 8], F32, kind="ExternalInput")
    outT = nc.dram_tensor("outT", [OUT, ROWS], F32, kind="ExternalOutput")

    with tile.TileContext(nc) as tc:
        with tc.tile_pool(name="dram", bufs=1, space="DRAM") as dram:
            dec = dram.tile([NT1, 128, 16, R1], BF16)  # [ctx; last]

            # Outer pool: survives both phases (wd1 streams in during A).
            with tc.tile_pool(name="pw", bufs=1) as pw:
                wd1_sb = pw.tile([128, 16, DEC_H], BF16)
                bd1_sb = pw.tile([128, 16], F32, tag="bd1")
                bd2_sb = pw.tile([128, 8], F32, tag="bd2")
                ones_f32 = pw.tile([128, 128], F32, tag="ones_f32")
                ones_sb = pw.tile([128, 128], BF16, tag="ones")

                # ================= Phase A =================
                with (
                    tc.tile_pool(name="paw", bufs=1) as paw,
                    tc.tile_pool(name="pax", bufs=2) as pax,
                    tc.tile_pool(name="pap", bufs=2) as pap,
                    tc.tile_pool(name="pad", bufs=2) as pad,
                    tc.tile_pool(name="pat", bufs=1) as pat,
                    tc.tile_pool(name="pas", bufs=1) as pas,
                    tc.tile_pool(name="psA", bufs=4, space="PSUM") as psA,
                    tc.tile_pool(name="psAl", bufs=3, space="PSUM") as psAl,
                ):
                    xts = {}

                    def load_xt(rt):
                        t = pax.tile([128, 12, R1], BF16, tag="xt", name="xt")
                        nc.sync.dma_start(t, xT.ap()[rt])
                        xts[rt] = t

                    nc.any.memset(ones_f32, 1.0)
                    nc.vector.tensor_copy(ones_sb, ones_f32)
                    # PE warmup: keep the HAM clock-gate busy while the
                    # first weights stream in (fp32: 4 cyc/row).
                    for _ in range(10):
                        wps = psA.tile([128, R1], F32, tag="mm", name="warm")
                        nc.tensor.matmul(wps[:, 0:128], ones_f32, ones_f32,
                                         start=True, stop=True)

                    # scalar ring: w1..w3, bh, wh (4 groups) — nothing
                    # else, so L2's FIFO-count wait covers only these.
                    w_sb = []
                    b_sb = []
                    for i in range(3):
                        w = paw.tile([128, 4, M], BF16, tag=f"w{i}",
                                     name=f"w{i}")
                        nc.scalar.dma_start(w, wT[i].ap())
                        b = paw.tile([128, 8], F32, tag=f"b{i}", name=f"b{i}")
                        nc.scalar.dma_start(b, bv[i].ap())
                        w_sb.append(w)
                        b_sb.append(b)
                        if i == 0:
                            load_xt(0)
                        if i == 1:
                            load_xt(1)
                    bh_sb = paw.tile([128, 8], F32, tag="bh", name="bh")
                    nc.scalar.dma_start(bh_sb, bh.ap())
                    wh_sb = paw.tile([128, 24, M], BF16, tag="wh", name="wh")
                    for g in range(4):
                        nc.scalar.dma_start(wh_sb[:, 6 * g:6 * g + 6, :],
                                            whT.ap()[:, 6 * g:6 * g + 6, :])
                    nc.gpsimd.dma_start(bd1_sb, bd1.ap())
                    nc.gpsimd.dma_start(bd2_sb, bd2.ap())

                    for rt in range(NT1):
                        xt = xts.pop(rt)

                        # ---- L1: P_i = relu(x_i @ w_i.T + b_i) ----
                        pt = []
                        for i in range(3):
                            p_i = pap.tile([128, 8, R1], BF16, tag=f"p{i}",
                                           name=f"p{i}")
                            for fc in range(8):
                                ps = psA.tile([128, R1], F32, tag="mm",
                                              name="ps1")
                                for kc in range(4):
                                    nc.tensor.matmul(
                                        ps,
                                        w_sb[i][:, kc, fc * 128:(fc + 1) * 128],
                                        xt[:, i * 4 + kc, :],
                                        start=(kc == 0),
                                        stop=(kc == 3),
                                    )
                                nc.scalar.activation(
                                    p_i[:, fc, :], ps, Act.Relu,
                                    bias=b_sb[i][:, fc:fc + 1],
                                )
                            pt.append(p_i)

                        if rt + 2 < NT1:
                            load_xt(rt + 2)
                        # wd1 stream, spread so it doesn't contend with wh
                        # for SDMA packet slots early on (gpsimd ring).
                        if 1 <= rt <= 5:
                            for kc in range(3 * (rt - 1), 3 * rt):
                                nc.gpsimd.dma_start(wd1_sb[:, kc, :],
                                                    wd1T.ap()[:, kc, :])
                        if rt == 6:
                            nc.gpsimd.dma_start(wd1_sb[:, 15, :],
                                                wd1T.ap()[:, 15, :])

                        # ---- L2: last = relu(hid1 @ wh.T + bh) ----
                        dec_sb = pad.tile([128, 16, R1], BF16, tag="dec",
                                          name="dec")
                        last = dec_sb[:, 8:16, :]
                        for fc in range(8):
                            ps = psA.tile([128, R1], F32, tag="mm", name="ps2")
                            for i in range(3):
                                for kc in range(8):
                                    nc.tensor.matmul(
                                        ps,
                                        wh_sb[:, i * 8 + kc,
                                              fc * 128:(fc + 1) * 128],
                                        pt[i][:, kc, :],
                                        start=(i == 0 and kc == 0),
                                        stop=(i == 2 and kc == 7),
                                    )
                            nc.scalar.activation(
                                last[:, fc, :], ps, Act.Relu,
                                bias=bh_sb[:, fc:fc + 1],
                            )

                        # ---- attention: alphas via bf16 ones-matmul ----
                        aps = []
                        for i in range(3):
                            tmp = pat.tile([128, 8, R1], BF16, tag="tmp",
                                           name=f"tmp{i}")
                            nc.vector.tensor_tensor(tmp, last, pt[i],
                                                    AluOp.mult)
                            ap_i = psAl.tile([128, R1], F32, tag="alpha",
                                             name=f"alpha{i}")
                            for fc in range(8):
                                nc.tensor.matmul(
                                    ap_i, ones_sb, tmp[:, fc, :],
                                    start=(fc == 0), stop=(fc == 7),
                                )
                            aps.append(ap_i)

                        # softmax over the 3 logits (fp32)
                        asb = pas.tile([128, 3, R1], F32, tag="asb")
                        for i in range(3):
                            nc.scalar.copy(asb[:, i, :], aps[i])
                        ai = asb.rearrange("p i r -> p r i")
                        mx = pas.tile([128, R1], F32, tag="mx")
                        nc.vector.reduce_max(mx, ai, axis=mybir.AxisListType.X)
                        bshp = (128, 3, R1)
                        nc.vector.tensor_tensor(
                            asb, asb, mx[:, None, :].to_broadcast(bshp),
                            AluOp.subtract)
                        nc.scalar.activation(asb, asb, Act.Exp)
                        ssum = pas.tile([128, R1], F32, tag="ssum")
                        nc.vector.reduce_sum(ssum, ai, axis=mybir.AxisListType.X)
                        rcp = pas.tile([128, R1], F32, tag="rcp")
                        nc.vector.reciprocal(rcp, ssum)
                        wsr = pas.tile([128, 3, R1], BF16, tag="wsr")
                        nc.vector.tensor_tensor(
                            wsr, asb, rcp[:, None, :].to_broadcast(bshp),
                            AluOp.mult)

                        # ctx = sum_i ws_i * P_i -> dec_sb[:, 0:8]
                        shp = (128, 8, R1)
                        ctx = dec_sb[:, 0:8, :]
                        nc.vector.tensor_tensor(
                            ctx, wsr[:, 0, None, :].to_broadcast(shp),
                            pt[0], AluOp.mult)
                        t2 = pat.tile([128, 8, R1], BF16, tag="tmp", name="t2")
                        nc.vector.tensor_tensor(
                            t2, wsr[:, 1, None, :].to_broadcast(shp),
                            pt[1], AluOp.mult)
                        nc.vector.tensor_tensor(ctx, ctx, t2, AluOp.add)
                        t3 = pat.tile([128, 8, R1], BF16, tag="tmp", name="t3")
                        nc.vector.tensor_tensor(
                            t3, wsr[:, 2, None, :].to_broadcast(shp),
                            pt[2], AluOp.mult)
                        nc.vector.tensor_tensor(ctx, ctx, t3, AluOp.add)

                        nc.gpsimd.dma_start(dec[rt], dec_sb)

                # ================= Phase B =================
                with (
                    # pbd first: lands on the SBUF region freed earliest.
                    tc.tile_pool(name="pbd", bufs=2) as pbd,
                    tc.tile_pool(name="pbo", bufs=2) as pbo,
                    tc.tile_pool(name="pbe", bufs=3) as pbe,
                    tc.tile_pool(name="pbw", bufs=1) as pbw,
                    tc.tile_pool(name="psD", bufs=3, space="PSUM") as psD,
                    tc.tile_pool(name="psE", bufs=3, space="PSUM") as psE,
                ):
                    dcs = {}

                    def load_dc(rt):
                        t = pbd.tile([128, 16, R1], BF16, tag="dc", name="dc")
                        nc.sync.dma_start(t, dec[rt])
                        dcs[rt] = t

                    load_dc(0)
                    load_dc(1)
                    # wd2 on gpsimd: behind the dec stores in that FIFO,
                    # ready well before the first D2 needs it.
                    wd2_sb = pbw.tile([128, 16, OUT], BF16, tag="wd2")
                    for g in range(4):
                        nc.gpsimd.dma_start(wd2_sb[:, 4 * g:4 * g + 4, :],
                                            wd2T.ap()[:, 4 * g:4 * g + 4, :])

                    for rt in range(NT1):
                        rs = slice(rt * R1, (rt + 1) * R1)
                        if rt + 2 < NT1:
                            load_dc(rt + 2)
                        dc = dcs.pop(rt)

                        o1 = pbo.tile([128, 16, R1], BF16, tag="o1",
                                      name="o1")
                        for fc in range(16):
                            ps = psD.tile([128, R1], F32, tag="d1")
                            for kc in range(16):
                                nc.tensor.matmul(
                                    ps,
                                    wd1_sb[:, kc, fc * 128:(fc + 1) * 128],
                                    dc[:, kc, :],
                                    start=(kc == 0),
                                    stop=(kc == 15),
                                )
                            nc.scalar.activation(
                                o1[:, fc, :], ps, Act.Relu,
                                bias=bd1_sb[:, fc:fc + 1],
                            )
                        for oc in range(8):
                            ow = 128 if oc < 7 else OUT - 7 * 128
                            ps = psE.tile([128, R1], F32, tag="d2")
                            for kc in range(16):
                                nc.tensor.matmul(
                                    ps[:ow],
                                    wd2_sb[:, kc, oc * 128:oc * 128 + ow],
                                    o1[:, kc, :],
                                    start=(kc == 0),
                                    stop=(kc == 15),
                                )
                            ev = pbe.tile([128, R1], F32, tag="ev")
                            nc.vector.tensor_scalar_add(
                                ev[:ow], ps[:ow], bd2_sb[:ow, oc:oc + 1]
                            )
                            eng = nc.sync if oc % 2 else nc.gpsimd
                            eng.dma_start(
                                outT.ap()[oc * 128:oc * 128 + ow, rs],
                                ev[:ow],
                            )

    nc.finalize()
    return nc


def _prep_inputs(tube, w1_W, w1_b, w2_W, w2_b, w3_W, w3_b, wh_W, wh_b,
                 wd1_W, wd1_b, wd2_W, wd2_b):
    """Host-side reshape/transpose into the kernel's DRAM layouts."""
    f32 = np.float32
    bf16 = ml_dtypes.bfloat16

    def wT(w, kc):  # [F, K] -> partition-major [128, kc, F], bf16
        w = np.asarray(w, f32)
        return np.ascontiguousarray(
            w.T.astype(bf16).reshape(kc, 128, w.shape[0]).transpose(1, 0, 2))

    def bmat(b, cc):  # [F] -> [128, cc]
        b = np.asarray(b, f32)
        if b.shape[0] < cc * 128:
            b = np.pad(b, (0, cc * 128 - b.shape[0]))
        return np.ascontiguousarray(b.reshape(cc, 128).T)

    shared = {
        "w1T": wT(w1_W, 4), "w2T": wT(w2_W, 4), "w3T": wT(w3_W, 4),
        "whT": wT(wh_W, 24), "wd1T": wT(wd1_W, 16), "wd2T": wT(wd2_W, 16),
        "b1": bmat(w1_b, 8), "b2": bmat(w2_b, 8), "b3": bmat(w3_b, 8),
        "bh": bmat(wh_b, 8), "bd1": bmat(wd1_b, 16), "bd2": bmat(wd2_b, 8),
    }
    tubeT = np.ascontiguousarray(np.asarray(tube, f32).T.astype(bf16))
    in_maps = []
    for c in range(N_CORES):
        # [1536, ROWS] -> tiled partition-major [NT1, 128, 12, R1]
        xTc = np.ascontiguousarray(
            tubeT[:, c * ROWS:(c + 1) * ROWS]
            .reshape(12, 128, NT1, R1).transpose(2, 1, 0, 3))
        in_maps.append({"xT": xTc, **shared})
    return in_maps


_NC_CACHE = {}


def run(inputs, mm_dtype=None, trace=False):
    # mm_dtype kept for test.py compat; the kernel is bf16-only now.
    if "nc" not in _NC_CACHE:
        _NC_CACHE["nc"] = build_nc()
    nc = _NC_CACHE["nc"]
    in_maps = _prep_inputs(**inputs)
    res = run_bass_kernel_spmd(nc, in_maps, list(range(N_CORES)), trace=trace)
    out = np.empty((B, OUT), np.float32)
    for c in range(N_CORES):
        out[c * ROWS:(c + 1) * ROWS] = res.results[c]["outT"].T
    return out, res


def kernel(**inputs) -> np.ndarray:
    out, _ = run(inputs)
    return out


# revision 13
# speedup vs baseline: 1.4728x; 1.0063x over previous
"""Trainium2 Bass kernel for nn_ClassAtt (dense MLP + 3-way class attention).

Model (per row of tube [B, 1536]):
  x1,x2,x3 = tube split into 3x512
  P_i   = relu(x_i @ w_i.T + b_i)            [B, 1024]
  last  = relu(concat(P1,P2,P3) @ wh.T + bh) [B, 1024]
  a_i   = rowwise_dot(last, P_i); w = softmax(a)  [B, 3]
  ctx   = sum_i w_i * P_i                    [B, 1024]
  out   = relu(concat(ctx, last) @ wd1.T + bd1) @ wd2.T + bd2  [B, 1000]

Strategy (v5): pure data parallel over 8 NeuronCores (2048 rows each).
All matmuls in bf16 (full PE rate, half the DMA/SBUF of fp32) with fp32
PSUM accumulation; rel err vs fp32 reference ~5.5e-3.  Activations in
transposed [feature, row] layout: contraction on SBUF partitions,
biases per-partition scalars.

Two fused phases, one DRAM spill (dec=[ctx;last]) between them; row
tiles of 256 in both.  A 10-matmul fp32 warmup at t=0 keeps the PE HAM
clock-gate warm through the initial weight load.

DMA: SDMA engines round-robin *packet slots* across active queues, so
per-queue bandwidth ~ packet size.  Every DRAM tensor is therefore
host-prearranged into its exact SBUF layout (partition-major) so each
transfer is a single DMA with multi-KB per-partition contiguity:
  xT [8,128,12,256] (one 6KB/part DMA per row tile, sync ring)
  w1/w2/w3 [128,4,1024], wh [128,24,1024] (scalar ring; wh in 4 groups
    so L2 tile 0 starts as chunks land)
  wd1 [128,16,2048] (gpsimd SWDGE, 3 chunks per row tile from tile 1 —
    off the scalar ring so L2's FIFO-count wait never covers it, and
    spread so it doesn't steal packet slots from wh)
  dec spill [8,128,16,256] A-tile-major (8KB/part contiguous both ways)
  out writes alternate sync/gpsimd to halve the final drain.
Phase-B pools open pbd first so dec loads land on the SBUF region
freed earliest by phase A (shortest WAR wait at the transition).
"""

import numpy as np
import ml_dtypes

import concourse.bass as bass
import concourse.mybir as mybir
import concourse.tile as tile
from concourse import bacc
from concourse.bass_utils import run_bass_kernel_spmd

F32 = mybir.dt.float32
F32R = mybir.dt.float32r
BF16 = mybir.dt.bfloat16

N_CORES = 8
B = 16384
ROWS = B // N_CORES  # rows per core
M = 1024             # hidden width
DEC_H = 2048
OUT = 1000
R1 = 256
NT1 = ROWS // R1

AluOp = mybir.AluOpType
Act = mybir.ActivationFunctionType


def build_nc():
    nc = bacc.Bacc(None, target_bir_lowering=False)

    # ---- DRAM I/O (per-core shapes, partition-major SBUF layouts) ----
    xT = nc.dram_tensor("xT", [NT1, 128, 12, R1], BF16, kind="ExternalInput")
    wT = [
        nc.dram_tensor(f"w{i + 1}T", [128, 4, M], BF16, kind="ExternalInput")
        for i in range(3)
    ]
    whT = nc.dram_tensor("whT", [128, 24, M], BF16, kind="ExternalInput")
    wd1T = nc.dram_tensor("wd1T", [128, 16, DEC_H], BF16, kind="ExternalInput")
    wd2T = nc.dram_tensor("wd2T", [128, 16, OUT], BF16, kind="ExternalInput")
    bv = [
        nc.dram_tensor(f"b{i + 1}", [128, 8], F32, kind="ExternalInput")
        for i in range(3)
    ]
    bh = nc.dram_tensor("bh", [128, 8], F32, kind="ExternalInput")
    bd1 = nc.dram_tensor("bd1", [128, 16], F32, kind="ExternalInput")
    bd2 = nc.dram_tensor("bd2", [128, 8], F32, kind="ExternalInput")
    outT = nc.dram_tensor("outT", [OUT, ROWS], F32, kind="ExternalOutput")

    with tile.TileContext(nc) as tc:
        with tc.tile_pool(name="dram", bufs=1, space="DRAM") as dram:
            dec = dram.tile([NT1, 128, 16, R1], BF16)  # [ctx; last]

            # Outer pool: survives both phases (wd1 streams in during A).
            with tc.tile_pool(name="pw", bufs=1) as pw:
                wd1_sb = pw.tile([128, 16, DEC_H], BF16)
                bd1_sb = pw.tile([128, 16], F32, tag="bd1")
                bd2_sb = pw.tile([128, 8], F32, tag="bd2")
                ones_f32 = pw.tile([128, 128], F32, tag="ones_f32")
                ones_sb = pw.tile([128, 128], BF16, tag="ones")

                # ================= Phase A =================
                with (
                    tc.tile_pool(name="paw", bufs=1) as paw,
                    tc.tile_pool(name="pax", bufs=2) as pax,
                    tc.tile_pool(name="pap", bufs=2) as pap,
                    tc.tile_pool(name="pad", bufs=2) as pad,
                    tc.tile_pool(name="pat", bufs=1) as pat,
                    tc.tile_pool(name="pas", bufs=1) as pas,
                    tc.tile_pool(name="psA", bufs=4, space="PSUM") as psA,
                    tc.tile_pool(name="psAl", bufs=3, space="PSUM") as psAl,
                ):
                    xts = {}

                    def load_xt(rt):
                        t = pax.tile([128, 12, R1], BF16, tag="xt", name="xt")
                        nc.sync.dma_start(t, xT.ap()[rt])
                        xts[rt] = t

                    nc.any.memset(ones_f32, 1.0)
                    nc.vector.tensor_copy(ones_sb, ones_f32)
                    # PE warmup: keep the HAM clock-gate busy while the
                    # first weights stream in (fp32: 4 cyc/row).
                    for _ in range(14):
                        wps = psA.tile([128, R1], F32, tag="mm", name="warm")
                        nc.tensor.matmul(wps[:, 0:128], ones_f32, ones_f32,
                                         start=True, stop=True)

                    # scalar ring: w1..w3, bh, wh (4 groups) — nothing
                    # else, so L2's FIFO-count wait covers only these.
                    w_sb = []
                    b_sb = []
                    for i in range(3):
                        w = paw.tile([128, 4, M], BF16, tag=f"w{i}",
                                     name=f"w{i}")
                        nc.scalar.dma_start(w, wT[i].ap())
                        b = paw.tile([128, 8], F32, tag=f"b{i}", name=f"b{i}")
                        nc.scalar.dma_start(b, bv[i].ap())
                        w_sb.append(w)
                        b_sb.append(b)
                        if i == 0:
                            load_xt(0)
                        if i == 1:
                            load_xt(1)
                    bh_sb = paw.tile([128, 8], F32, tag="bh", name="bh")
                    nc.scalar.dma_start(bh_sb, bh.ap())
                    wh_sb = paw.tile([128, 24, M], BF16, tag="wh", name="wh")
                    for g in range(4):
                        nc.scalar.dma_start(wh_sb[:, 6 * g:6 * g + 6, :],
                                            whT.ap()[:, 6 * g:6 * g + 6, :])
                    nc.gpsimd.dma_start(bd1_sb, bd1.ap())
                    nc.gpsimd.dma_start(bd2_sb, bd2.ap())

                    for rt in range(NT1):
                        xt = xts.pop(rt)

                        # ---- L1: P_i = relu(x_i @ w_i.T + b_i) ----
                        pt = []
                        for i in range(3):
                            p_i = pap.tile([128, 8, R1], BF16, tag=f"p{i}",
                                           name=f"p{i}")
                            for fc in range(8):
                                ps = psA.tile([128, R1], F32, tag="mm",
                                              name="ps1")
                                for kc in range(4):
                                    nc.tensor.matmul(
                                        ps,
                                        w_sb[i][:, kc, fc * 128:(fc + 1) * 128],
                                        xt[:, i * 4 + kc, :],
                                        start=(kc == 0),
                                        stop=(kc == 3),
                                    )
                                nc.scalar.activation(
                                    p_i[:, fc, :], ps, Act.Relu,
                                    bias=b_sb[i][:, fc:fc + 1],
                                )
                            pt.append(p_i)

                        # wd1 stream, spread so it doesn't contend with wh
                        # for SDMA packet slots early on (gpsimd ring).
                        if 1 <= rt <= 5:
                            for kc in range(3 * (rt - 1), 3 * rt):
                                nc.gpsimd.dma_start(wd1_sb[:, kc, :],
                                                    wd1T.ap()[:, kc, :])
                        if rt == 6:
                            nc.gpsimd.dma_start(wd1_sb[:, 15, :],
                                                wd1T.ap()[:, 15, :])

                        # ---- L2: last = relu(hid1 @ wh.T + bh) ----
                        dec_sb = pad.tile([128, 16, R1], BF16, tag="dec",
                                          name="dec")
                        last = dec_sb[:, 8:16, :]
                        for fc in range(8):
                            ps = psA.tile([128, R1], F32, tag="mm", name="ps2")
                            for i in range(3):
                                for kc in range(8):
                                    nc.tensor.matmul(
                                        ps,
                                        wh_sb[:, i * 8 + kc,
                                              fc * 128:(fc + 1) * 128],
                                        pt[i][:, kc, :],
                                        start=(i == 0 and kc == 0),
                                        stop=(i == 2 and kc == 7),
                                    )
                            nc.scalar.activation(
                                last[:, fc, :], ps, Act.Relu,
                                bias=bh_sb[:, fc:fc + 1],
                            )

                        # ---- attention: alphas via bf16 ones-matmul ----
                        aps = []
                        for i in range(3):
                            tmp = pat.tile([128, 8, R1], BF16, tag="tmp",
                                           name=f"tmp{i}")
                            nc.vector.tensor_tensor(tmp, last, pt[i],
                                                    AluOp.mult)
                            ap_i = psAl.tile([128, R1], F32, tag="alpha",
                                             name=f"alpha{i}")
                            for fc in range(8):
                                nc.tensor.matmul(
                                    ap_i, ones_sb, tmp[:, fc, :],
                                    start=(fc == 0), stop=(fc == 7),
                                )
                            aps.append(ap_i)

                        # softmax over the 3 logits (fp32)
                        asb = pas.tile([128, 3, R1], F32, tag="asb")
                        for i in range(3):
                            nc.scalar.copy(asb[:, i, :], aps[i])
                        ai = asb.rearrange("p i r -> p r i")
                        mx = pas.tile([128, R1], F32, tag="mx")
                        nc.vector.reduce_max(mx, ai, axis=mybir.AxisListType.X)
                        bshp = (128, 3, R1)
                        nc.vector.tensor_tensor(
                            asb, asb, mx[:, None, :].to_broadcast(bshp),
                            AluOp.subtract)
                        nc.scalar.activation(asb, asb, Act.Exp)
                        ssum = pas.tile([128, R1], F32, tag="ssum")
                        nc.vector.reduce_sum(ssum, ai, axis=mybir.AxisListType.X)
                        rcp = pas.tile([128, R1], F32, tag="rcp")
                        nc.vector.reciprocal(rcp, ssum)
                        wsr = pas.tile([128, 3, R1], BF16, tag="wsr")
                        nc.vector.tensor_tensor(
                            wsr, asb, rcp[:, None, :].to_broadcast(bshp),
                            AluOp.mult)

                        # ctx = sum_i ws_i * P_i -> dec_sb[:, 0:8]
                        shp = (128, 8, R1)
                        ctx = dec_sb[:, 0:8, :]
                        nc.vector.tensor_tensor(
                            ctx, wsr[:, 0, None, :].to_broadcast(shp),
                            pt[0], AluOp.mult)
                        t2 = pat.tile([128, 8, R1], BF16, tag="tmp", name="t2")
                        nc.vector.tensor_tensor(
                            t2, wsr[:, 1, None, :].to_broadcast(shp),
                            pt[1], AluOp.mult)
                        nc.vector.tensor_tensor(ctx, ctx, t2, AluOp.add)
                        t3 = pat.tile([128, 8, R1], BF16, tag="tmp", name="t3")
                        nc.vector.tensor_tensor(
                            t3, wsr[:, 2, None, :].to_broadcast(shp),
                            pt[2], AluOp.mult)
                        nc.vector.tensor_tensor(ctx, ctx, t3, AluOp.add)

                        nc.gpsimd.dma_start(dec[rt], dec_sb)

                        # xt prefetch emitted LAST: keeps its packets out
                        # of the slot-rotation while wh/earlier tiles are
                        # still streaming (needed ~60us later anyway).
                        if rt + 2 < NT1:
                            load_xt(rt + 2)

                # ================= Phase B =================
                with (
                    # pbd first: lands on the SBUF region freed earliest.
                    tc.tile_pool(name="pbd", bufs=2) as pbd,
                    tc.tile_pool(name="pbo", bufs=2) as pbo,
                    tc.tile_pool(name="pbe", bufs=3) as pbe,
                    tc.tile_pool(name="pbw", bufs=1) as pbw,
                    tc.tile_pool(name="psD", bufs=3, space="PSUM") as psD,
                    tc.tile_pool(name="psE", bufs=3, space="PSUM") as psE,
                ):
                    dcs = {}

                    def load_dc(rt):
                        t = pbd.tile([128, 16, R1], BF16, tag="dc", name="dc")
                        nc.sync.dma_start(t, dec[rt])
                        dcs[rt] = t

                    load_dc(0)
                    load_dc(1)
                    # wd2 on gpsimd: behind the dec stores in that FIFO,
                    # ready well before the first D2 needs it.
                    wd2_sb = pbw.tile([128, 16, OUT], BF16, tag="wd2")
                    for g in range(4):
                        nc.gpsimd.dma_start(wd2_sb[:, 4 * g:4 * g + 4, :],
                                            wd2T.ap()[:, 4 * g:4 * g + 4, :])

                    for rt in range(NT1):
                        rs = slice(rt * R1, (rt + 1) * R1)
                        if rt + 2 < NT1:
                            load_dc(rt + 2)
                        dc = dcs.pop(rt)

                        o1 = pbo.tile([128, 16, R1], BF16, tag="o1",
                                      name="o1")
                        for fc in range(16):
                            ps = psD.tile([128, R1], F32, tag="d1")
                            for kc in range(16):
                                nc.tensor.matmul(
                                    ps,
                                    wd1_sb[:, kc, fc * 128:(fc + 1) * 128],
                                    dc[:, kc, :],
                                    start=(kc == 0),
                                    stop=(kc == 15),
                                )
                            nc.scalar.activation(
                                o1[:, fc, :], ps, Act.Relu,
                                bias=bd1_sb[:, fc:fc + 1],
                            )
                        for oc in range(8):
                            ow = 128 if oc < 7 else OUT - 7 * 128
                            ps = psE.tile([128, R1], F32, tag="d2")
                            for kc in range(16):
                                nc.tensor.matmul(
                                    ps[:ow],
                                    wd2_sb[:, kc, oc * 128:oc * 128 + ow],
                                    o1[:, kc, :],
                                    start=(kc == 0),
                                    stop=(kc == 15),
                                )
                            ev = pbe.tile([128, R1], F32, tag="ev")
                            nc.vector.tensor_scalar_add(
                                ev[:ow], ps[:ow], bd2_sb[:ow, oc:oc + 1]
                            )
                            eng = nc.sync if oc % 2 else nc.gpsimd
                            eng.dma_start(
                                outT.ap()[oc * 128:oc * 128 + ow, rs],
                                ev[:ow],
                            )

    nc.finalize()
    return nc


def _prep_inputs(tube, w1_W, w1_b, w2_W, w2_b, w3_W, w3_b, wh_W, wh_b,
                 wd1_W, wd1_b, wd2_W, wd2_b):
    """Host-side reshape/transpose into the kernel's DRAM layouts."""
    f32 = np.float32
    bf16 = ml_dtypes.bfloat16

    def wT(w, kc):  # [F, K] -> partition-major [128, kc, F], bf16
        w = np.asarray(w, f32)
        return np.ascontiguousarray(
            w.T.astype(bf16).reshape(kc, 128, w.shape[0]).transpose(1, 0, 2))

    def bmat(b, cc):  # [F] -> [128, cc]
        b = np.asarray(b, f32)
        if b.shape[0] < cc * 128:
            b = np.pad(b, (0, cc * 128 - b.shape[0]))
        return np.ascontiguousarray(b.reshape(cc, 128).T)

    shared = {
        "w1T": wT(w1_W, 4), "w2T": wT(w2_W, 4), "w3T": wT(w3_W, 4),
        "whT": wT(wh_W, 24), "wd1T": wT(wd1_W, 16), "wd2T": wT(wd2_W, 16),
        "b1": bmat(w1_b, 8), "b2": bmat(w2_b, 8), "b3": bmat(w3_b, 8),
        "bh": bmat(wh_b, 8), "bd1": bmat(wd1_b, 16), "bd2": bmat(wd2_b, 8),
    }
    tubeT = np.ascontiguousarray(np.asarray(tube, f32).T.astype(bf16))
    in_maps = []
    for c in range(N_CORES):
        # [1536, ROWS] -> tiled partition-major [NT1, 128, 12, R1]
        xTc = np.ascontiguousarray(
            tubeT[:, c * ROWS:(c + 1) * ROWS]
            .reshape(12, 128, NT1, R1).transpose(2, 1, 0, 3))
        in_maps.append({"xT": xTc, **shared})
    return in_maps


_NC_CACHE = {}


def run(inputs, mm_dtype=None, trace=False):
    # mm_dtype kept for test.py compat; the kernel is bf16-only now.
    if "nc" not in _NC_CACHE:
        _NC_CACHE["nc"] = build_nc()
    nc = _NC_CACHE["nc"]
    in_maps = _prep_inputs(**inputs)
    res = run_bass_kernel_spmd(nc, in_maps, list(range(N_CORES)), trace=trace)
    out = np.empty((B, OUT), np.float32)
    for c in range(N_CORES):
        out[c * ROWS:(c + 1) * ROWS] = res.results[c]["outT"].T
    return out, res


def kernel(**inputs) -> np.ndarray:
    out, _ = run(inputs)
    return out
